# revision 6
# baseline (speedup 1.0000x reference)
"""CorrelationSampler Trainium2 kernel — band-matmul formulation.

out[b, p, c] = sum of 4 bilinear corner weights * corr[b, corner_row(p), c]
            = (S_b @ corr_b)[p, c]

where S_b is a [4096, 4096] sparse matrix with 4 nonzeros per row at
columns {r, r+1, r+64, r+65} (r = iy0*64+ix0 per output position).

Key idea: the naive gather reads every correlation row ~4x (4 corners per
position, rows shared between positions). Casting to bf16 and sorting the
output positions by base row r makes S band-diagonal: each 128-position
tile only touches ~2-3 adjacent 128-row tiles of corr. The TensorEngine
then computes out = S @ corr with corr streamed from HBM exactly ONCE.

Per-core HBM traffic drops from ~160 MB (fp32 gather baseline) to ~35 MB:
  16 MB corr band (bf16) + 3 MB S blocks + 16 MB output (bf16).

Sharding: 8 cores = 4 batches x 2 channel-halves (each core: all 4096
positions of one batch, 2048 of the 4096 channels). S depends only on
flow, so the two halves of a batch share the same S. bf16 is safe: the
tolerance is 2e-2 and bf16 end-to-end error is ~0.5e-2 worst-case.
"""

import numpy as np
import ml_dtypes

BF16 = np.dtype(ml_dtypes.bfloat16)

B, H, W = 4, 64, 64
HW = H * W  # 4096
N_CORES = 8
P = 128
N_PTILES = HW // P  # 32 position tiles (all positions, sorted)
CH_PER_CORE = HW // 2  # 2048 channels per core
N_CHUNK = 512  # matmul free dim (one PSUM bank of fp32)
N_CHUNKS = CH_PER_CORE // N_CHUNK  # 4
STORE_GROUP = 4  # pos-tiles buffered per output store (512 KB stores)
SUB = 8  # source tiles per band sub-load (1 MB sub-loads)
N_SUBS = N_PTILES // SUB  # 4 band sub-tiles per chunk


def _host_indices_weights(flow: np.ndarray):
    """float32 replica of the reference's grid math -> base corner row
    index row0 and the 4 bilinear corner weights, shape [B, H*W] each.
    Corner rows of position p are row0, row0+1, row0+64, row0+65."""
    f32 = np.float32
    y_g, x_g = np.meshgrid(
        np.arange(H, dtype=f32), np.arange(W, dtype=f32), indexing="ij"
    )
    x_norm = (f32(2.0) * x_g / f32(W - 1) - f32(1.0)).astype(f32)
    y_norm = (f32(2.0) * y_g / f32(H - 1) - f32(1.0)).astype(f32)

    fx = flow[:, 0].astype(f32)
    fy = flow[:, 1].astype(f32)
    gx = x_norm[None] + fx / f32(W) * f32(2.0)
    gy = y_norm[None] + fy / f32(H) * f32(2.0)

    ix = np.clip((gx + f32(1.0)) * f32(0.5) * f32(W - 1), f32(0.0), f32(W - 1))
    iy = np.clip((gy + f32(1.0)) * f32(0.5) * f32(H - 1), f32(0.0), f32(H - 1))

    # floor >= 0 after the clip; clamp to W-2/H-2 so the +1 neighbor always
    # exists (at the high border all weight lands on the last row/col --
    # identical to the reference's clip formulation).
    ix0 = np.minimum(np.floor(ix), f32(W - 2)).astype(np.int32)
    iy0 = np.minimum(np.floor(iy), f32(H - 2)).astype(np.int32)
    wx = (ix - ix0.astype(f32)).astype(f32)
    wy = (iy - iy0.astype(f32)).astype(f32)

    one = f32(1.0)
    w00 = ((one - wy) * (one - wx)).astype(f32)
    w01 = ((one - wy) * wx).astype(f32)
    w10 = (wy * (one - wx)).astype(f32)
    w11 = (wy * wx).astype(f32)

    row0 = iy0 * np.int32(W) + ix0
    flat = lambda a: a.reshape(B, HW)
    return flat(row0), flat(w00), flat(w01), flat(w10), flat(w11)


def _windows(radius):
    """Static per-pos-tile source-tile windows (flow independent so all 8
    SPMD cores share one program)."""
    wins = []
    for k in range(N_PTILES):
        wins.append(list(range(max(0, k - radius), min(N_PTILES - 1, k + radius) + 1)))
    return wins


def _build_program(radius):
    import concourse.bacc as bacc
    import concourse.mybir as mybir
    from concourse.tile import TileContext

    bf16 = mybir.dt.bfloat16
    f32 = mybir.dt.float32

    wins = _windows(radius)
    nmm = sum(len(w) for w in wins)

    nc = bacc.Bacc(
        "TRN2", target_bir_lowering=False, debug=False, num_devices=N_CORES
    )
    # band[p, s, ch]: corr row s*128+p, channel ch (this core's half)
    band = nc.dram_tensor(
        "band", [P, N_PTILES, CH_PER_CORE], bf16, kind="ExternalInput"
    ).ap()
    # s_all[:, j*128:(j+1)*128] = j-th stationary block S^T[src_row, pos]
    s_all = nc.dram_tensor(
        "s_all", [P, nmm * P], bf16, kind="ExternalInput"
    ).ap()
    # out[p, k, ch]: sorted position k*128+p
    out = nc.dram_tensor(
        "out", [P, N_PTILES, CH_PER_CORE], bf16, kind="ExternalOutput"
    ).ap()

    # k-major loop: for each sorted-position tile k, run all 4 channel
    # chunks per stationary block (stationary reused across chunks), one
    # [128, 2048] psum evacuation + one 4KB-contiguous store per k.
    # Band source tiles stream ascending (tile s is dead after k = s+1),
    # so a handful of 1 MB buffers gives deep prefetch; every DMA moves
    # 4KB-contiguous runs per partition.
    n_pieces = 8
    bounds = [round(i * nmm / n_pieces) for i in range(n_pieces + 1)]
    # flat block index -> S piece
    blk_piece = np.searchsorted(bounds, np.arange(nmm), side="right") - 1
    # first flat block per k (blocks consumed in k order)
    k_flat0 = np.cumsum([0] + [len(w) for w in wins])

    with TileContext(nc) as tc:
        with (
            tc.tile_pool(name="meta", bufs=1) as meta,
            tc.tile_pool(name="bandp", bufs=8) as bandp,
            tc.tile_pool(name="outp", bufs=3) as outp,
            tc.tile_pool(name="psum", bufs=2, space="PSUM") as psump,
        ):
            s_ts = [None] * n_pieces
            band_ts = [None] * N_PTILES
            next_s = 0
            next_b = 0

            def load_s(pi):
                npc = bounds[pi + 1] - bounds[pi]
                st = meta.tile([P, npc * P], bf16, tag=f"s{pi}")
                nc.sync.dma_start(
                    out=st[:], in_=s_all[:, bounds[pi] * P : bounds[pi + 1] * P]
                )
                s_ts[pi] = st

            def load_band(s):
                bt = bandp.tile([P, CH_PER_CORE], bf16, tag="band")
                nc.sync.dma_start(out=bt[:], in_=band[:, s, :])
                band_ts[s] = bt

            for k in range(N_PTILES):
                win = wins[k]
                # issue loads just-in-time in program order: everything
                # this k needs, plus nothing (the pool depth provides the
                # prefetch run-ahead)
                while next_s < n_pieces and bounds[next_s] < k_flat0[k + 1]:
                    load_s(next_s)
                    next_s += 1
                while next_b <= min(win[-1], N_PTILES - 1):
                    load_band(next_b)
                    next_b += 1

                ps = psump.tile([P, CH_PER_CORE], f32, tag="ps")
                for i, s in enumerate(win):
                    j = int(k_flat0[k]) + i
                    pi = int(blk_piece[j])
                    loc = j - bounds[pi]
                    for c in range(N_CHUNKS):
                        nc.tensor.matmul(
                            ps[:, c * N_CHUNK : (c + 1) * N_CHUNK],
                            s_ts[pi][:, loc * P : (loc + 1) * P],
                            band_ts[s][:, c * N_CHUNK : (c + 1) * N_CHUNK],
                            start=(i == 0),
                            stop=(i == len(win) - 1),
                        )
                ot = outp.tile([P, CH_PER_CORE], bf16, tag="out")
                nc.vector.tensor_copy(out=ot[:], in_=ps[:])
                nc.sync.dma_start(out=out[:, k, :], in_=ot[:])
    nc.compile()
    return nc


def _plan(flow):
    """Sort positions by base row per batch, pick the minimal window
    radius covering every corner, and build the S^T stationary blocks."""
    row0, w00, w01, w10, w11 = _host_indices_weights(flow)

    perms = []
    row0s_list = []
    radius = 1
    for b in range(B):
        perm = np.argsort(row0[b], kind="stable")
        perms.append(perm)
        r0s = row0[b][perm]
        row0s_list.append(r0s)
        tiles = r0s.reshape(N_PTILES, P)
        lo = tiles.min(axis=1)  # min corner row per pos-tile
        hi = tiles.max(axis=1) + W + 1  # max corner row (row0+65)
        k = np.arange(N_PTILES)
        need = max(int(np.max(k - lo // P)), int(np.max(hi // P - k)))
        radius = max(radius, need)

    wins = _windows(radius)
    nmm = sum(len(w) for w in wins)

    s_blocks = []  # per batch: [P, nmm*P] bf16
    for b in range(B):
        r0s = row0s_list[b]
        ws = [a[b][perms[b]] for a in (w00, w01, w10, w11)]
        corners = [r0s, r0s + 1, r0s + W, r0s + W + 1]
        s_all = np.zeros((P, nmm * P), dtype=np.float32)
        flat = 0
        pos_local = np.tile(np.arange(P), N_PTILES).reshape(N_PTILES, P)
        for k in range(N_PTILES):
            sl = slice(k * P, (k + 1) * P)
            for i, s in enumerate(wins[k]):
                blk = s_all[:, (flat + i) * P : (flat + i + 1) * P]
                base = s * P
                for g_all, w_all in zip(corners, ws):
                    g = g_all[sl] - base
                    w = w_all[sl]
                    m = (g >= 0) & (g < P)
                    np.add.at(blk, (g[m], pos_local[k][m]), w[m])
            flat += len(wins[k])
        s_blocks.append(np.ascontiguousarray(s_all.astype(BF16)))

    return radius, perms, s_blocks


_cached = {}


def _get_program(radius):
    key = ("nc", radius)
    if key not in _cached:
        _cached[key] = _build_program(radius)
    return _cached[key]


def _ensure_axon_hooks_importable():
    """bass_utils imports antenv.axon_hooks when tracing is requested.
    Some containers ship an antenv stub without that module; provide a
    no-op registry so tracing degrades gracefully instead of crashing."""
    import sys
    import types

    try:
        import antenv.axon_hooks  # noqa: F401
    except Exception:
        m = types.ModuleType("antenv.axon_hooks")
        m._hook = None
        m.set_axon_ntff_profile_hook = lambda h: setattr(m, "_hook", h)
        m.get_axon_ntff_profile_hook = lambda: getattr(m, "_hook", None)
        sys.modules["antenv.axon_hooks"] = m


def kernel(correlation: np.ndarray, flow: np.ndarray, _trace: bool = False):
    _ensure_axon_hooks_importable()
    from concourse.bass_utils import run_bass_kernel_spmd

    correlation = np.asarray(correlation, dtype=np.float32)
    flow = np.asarray(flow, dtype=np.float32)

    radius, perms, s_blocks = _plan(flow)

    in_maps = []
    for core in range(N_CORES):
        b, half = divmod(core, 2)
        ch0 = half * CH_PER_CORE
        # band[p, s, ch] = corr[b, s*128+p, ch0+ch]
        band = (
            correlation[b]
            .reshape(HW, HW)[:, ch0 : ch0 + CH_PER_CORE]
            .astype(BF16)
            .reshape(N_PTILES, P, CH_PER_CORE)
            .transpose(1, 0, 2)
        )
        in_maps.append(
            {
                "band": np.ascontiguousarray(band),
                "s_all": s_blocks[b],
            }
        )

    nc = _get_program(radius)
    extra = {"trace_cores": list(range(N_CORES))} if _trace else {}
    res = run_bass_kernel_spmd(
        nc, in_maps, core_ids=list(range(N_CORES)), trace=_trace, **extra
    )

    out = np.empty((B, HW, HW), dtype=np.float32)
    for b in range(B):
        halves = [
            np.asarray(res.results[2 * b + h]["out"]).astype(np.float32)
            for h in range(2)
        ]
        # [P, N_PTILES, 4096] -> sorted-position-major [4096, 4096]
        full = np.concatenate(halves, axis=2).transpose(1, 0, 2).reshape(HW, HW)
        out[b, perms[b], :] = full
    if _trace:
        kernel.last_results = res
    return out.reshape(B, H, W, HW)


# revision 8
# speedup vs baseline: 1.1671x; 1.1671x over previous
"""CorrelationSampler Trainium2 kernel — band-matmul formulation.

out[b, p, c] = sum of 4 bilinear corner weights * corr[b, corner_row(p), c]
            = (S_b @ corr_b)[p, c]

where S_b is a [4096, 4096] sparse matrix with 4 nonzeros per row at
columns {r, r+1, r+64, r+65} (r = iy0*64+ix0 per output position).

Key idea: the naive gather reads every correlation row ~4x (4 corners per
position, rows shared between positions). Casting to bf16 and sorting the
output positions by base row r makes S band-diagonal: each 128-position
tile only touches ~2-3 adjacent 128-row tiles of corr. The TensorEngine
then computes out = S @ corr with corr streamed from HBM exactly ONCE.

Per-core HBM traffic drops from ~160 MB (fp32 gather baseline) to ~35 MB:
  16 MB corr band (bf16) + 3 MB S blocks + 16 MB output (bf16).

Sharding: 8 cores = 4 batches x 2 channel-halves (each core: all 4096
positions of one batch, 2048 of the 4096 channels). S depends only on
flow, so the two halves of a batch share the same S. bf16 is safe: the
tolerance is 2e-2 and bf16 end-to-end error is ~0.5e-2 worst-case.
"""

import numpy as np
import ml_dtypes

BF16 = np.dtype(ml_dtypes.bfloat16)

B, H, W = 4, 64, 64
HW = H * W  # 4096
N_CORES = 8
P = 128
N_PTILES = HW // P  # 32 position tiles (all positions, sorted)
CH_PER_CORE = HW // 2  # 2048 channels per core
N_CHUNK = 512  # matmul free dim (one PSUM bank of fp32)
N_CHUNKS = CH_PER_CORE // N_CHUNK  # 4
STORE_GROUP = 4  # pos-tiles buffered per output store (512 KB stores)
SUB = 8  # source tiles per band sub-load (1 MB sub-loads)
N_SUBS = N_PTILES // SUB  # 4 band sub-tiles per chunk


def _host_indices_weights(flow: np.ndarray):
    """float32 replica of the reference's grid math -> base corner row
    index row0 and the 4 bilinear corner weights, shape [B, H*W] each.
    Corner rows of position p are row0, row0+1, row0+64, row0+65."""
    f32 = np.float32
    y_g, x_g = np.meshgrid(
        np.arange(H, dtype=f32), np.arange(W, dtype=f32), indexing="ij"
    )
    x_norm = (f32(2.0) * x_g / f32(W - 1) - f32(1.0)).astype(f32)
    y_norm = (f32(2.0) * y_g / f32(H - 1) - f32(1.0)).astype(f32)

    fx = flow[:, 0].astype(f32)
    fy = flow[:, 1].astype(f32)
    gx = x_norm[None] + fx / f32(W) * f32(2.0)
    gy = y_norm[None] + fy / f32(H) * f32(2.0)

    ix = np.clip((gx + f32(1.0)) * f32(0.5) * f32(W - 1), f32(0.0), f32(W - 1))
    iy = np.clip((gy + f32(1.0)) * f32(0.5) * f32(H - 1), f32(0.0), f32(H - 1))

    # floor >= 0 after the clip; clamp to W-2/H-2 so the +1 neighbor always
    # exists (at the high border all weight lands on the last row/col --
    # identical to the reference's clip formulation).
    ix0 = np.minimum(np.floor(ix), f32(W - 2)).astype(np.int32)
    iy0 = np.minimum(np.floor(iy), f32(H - 2)).astype(np.int32)
    wx = (ix - ix0.astype(f32)).astype(f32)
    wy = (iy - iy0.astype(f32)).astype(f32)

    one = f32(1.0)
    w00 = ((one - wy) * (one - wx)).astype(f32)
    w01 = ((one - wy) * wx).astype(f32)
    w10 = (wy * (one - wx)).astype(f32)
    w11 = (wy * wx).astype(f32)

    row0 = iy0 * np.int32(W) + ix0
    flat = lambda a: a.reshape(B, HW)
    return flat(row0), flat(w00), flat(w01), flat(w10), flat(w11)


def _windows(radius):
    """Static per-pos-tile source-tile windows (flow independent so all 8
    SPMD cores share one program)."""
    wins = []
    for k in range(N_PTILES):
        wins.append(list(range(max(0, k - radius), min(N_PTILES - 1, k + radius) + 1)))
    return wins


def _build_program(radius):
    import concourse.bacc as bacc
    import concourse.mybir as mybir
    from concourse.tile import TileContext

    bf16 = mybir.dt.bfloat16
    f32 = mybir.dt.float32

    wins = _windows(radius)
    nmm = sum(len(w) for w in wins)

    nc = bacc.Bacc(
        "TRN2", target_bir_lowering=False, debug=False, num_devices=N_CORES
    )
    # band[p, s, ch]: corr row s*128+p, channel ch (this core's half)
    band = nc.dram_tensor(
        "band", [P, N_PTILES, CH_PER_CORE], bf16, kind="ExternalInput"
    ).ap()
    # s_all[:, j*128:(j+1)*128] = j-th stationary block S^T[src_row, pos]
    s_all = nc.dram_tensor(
        "s_all", [P, nmm * P], bf16, kind="ExternalInput"
    ).ap()
    # out[p, k, ch]: sorted position k*128+p
    out = nc.dram_tensor(
        "out", [P, N_PTILES, CH_PER_CORE], bf16, kind="ExternalOutput"
    ).ap()

    # k-major loop: for each sorted-position tile k, run all 4 channel
    # chunks per stationary block (stationary reused across chunks), one
    # [128, 2048] psum evacuation + one 4KB-contiguous store per k.
    # Band source tiles stream ascending (tile s is dead after k = s+1),
    # so a handful of 1 MB buffers gives deep prefetch; every DMA moves
    # 4KB-contiguous runs per partition.
    n_pieces = 8
    bounds = [round(i * nmm / n_pieces) for i in range(n_pieces + 1)]
    # flat block index -> S piece
    blk_piece = np.searchsorted(bounds, np.arange(nmm), side="right") - 1
    # first flat block per k (blocks consumed in k order)
    k_flat0 = np.cumsum([0] + [len(w) for w in wins])

    with TileContext(nc) as tc:
        with (
            tc.tile_pool(name="meta", bufs=1) as meta,
            tc.tile_pool(name="bandp", bufs=8) as bandp,
            tc.tile_pool(name="outp", bufs=3) as outp,
            tc.tile_pool(name="psum", bufs=2, space="PSUM") as psump,
        ):
            s_ts = [None] * n_pieces
            band_ts = [None] * N_PTILES
            next_s = 0
            next_b = 0

            def load_s(pi):
                npc = bounds[pi + 1] - bounds[pi]
                st = meta.tile([P, npc * P], bf16, tag=f"s{pi}")
                # SWDGE path: keeps the HWDGE rings free for band/out
                nc.gpsimd.dma_start(
                    out=st[:], in_=s_all[:, bounds[pi] * P : bounds[pi + 1] * P]
                )
                s_ts[pi] = st

            def load_band(s):
                bt = bandp.tile([P, CH_PER_CORE], bf16, tag="band")
                nc.sync.dma_start(out=bt[:], in_=band[:, s, :])
                band_ts[s] = bt

            for k in range(N_PTILES):
                win = wins[k]
                # issue loads just-in-time in program order: everything
                # this k needs, plus nothing (the pool depth provides the
                # prefetch run-ahead)
                while next_s < n_pieces and bounds[next_s] < k_flat0[k + 1]:
                    load_s(next_s)
                    next_s += 1
                while next_b <= min(win[-1], N_PTILES - 1):
                    load_band(next_b)
                    next_b += 1

                ps = psump.tile([P, CH_PER_CORE], f32, tag="ps")
                for i, s in enumerate(win):
                    j = int(k_flat0[k]) + i
                    pi = int(blk_piece[j])
                    loc = j - bounds[pi]
                    for c in range(N_CHUNKS):
                        nc.tensor.matmul(
                            ps[:, c * N_CHUNK : (c + 1) * N_CHUNK],
                            s_ts[pi][:, loc * P : (loc + 1) * P],
                            band_ts[s][:, c * N_CHUNK : (c + 1) * N_CHUNK],
                            start=(i == 0),
                            stop=(i == len(win) - 1),
                        )
                ot = outp.tile([P, CH_PER_CORE], bf16, tag="out")
                nc.vector.tensor_copy(out=ot[:], in_=ps[:])
                # stores go on the other HWDGE ring (ACT) so a store
                # waiting on compute never head-of-line-blocks band loads
                nc.scalar.dma_start(out=out[:, k, :], in_=ot[:])
    nc.compile()
    return nc


def _plan(flow):
    """Sort positions by base row per batch, pick the minimal window
    radius covering every corner, and build the S^T stationary blocks."""
    row0, w00, w01, w10, w11 = _host_indices_weights(flow)

    perms = []
    row0s_list = []
    radius = 1
    for b in range(B):
        perm = np.argsort(row0[b], kind="stable")
        perms.append(perm)
        r0s = row0[b][perm]
        row0s_list.append(r0s)
        tiles = r0s.reshape(N_PTILES, P)
        lo = tiles.min(axis=1)  # min corner row per pos-tile
        hi = tiles.max(axis=1) + W + 1  # max corner row (row0+65)
        k = np.arange(N_PTILES)
        need = max(int(np.max(k - lo // P)), int(np.max(hi // P - k)))
        radius = max(radius, need)

    wins = _windows(radius)
    nmm = sum(len(w) for w in wins)

    s_blocks = []  # per batch: [P, nmm*P] bf16
    for b in range(B):
        r0s = row0s_list[b]
        ws = [a[b][perms[b]] for a in (w00, w01, w10, w11)]
        corners = [r0s, r0s + 1, r0s + W, r0s + W + 1]
        s_all = np.zeros((P, nmm * P), dtype=np.float32)
        flat = 0
        pos_local = np.tile(np.arange(P), N_PTILES).reshape(N_PTILES, P)
        for k in range(N_PTILES):
            sl = slice(k * P, (k + 1) * P)
            for i, s in enumerate(wins[k]):
                blk = s_all[:, (flat + i) * P : (flat + i + 1) * P]
                base = s * P
                for g_all, w_all in zip(corners, ws):
                    g = g_all[sl] - base
                    w = w_all[sl]
                    m = (g >= 0) & (g < P)
                    np.add.at(blk, (g[m], pos_local[k][m]), w[m])
            flat += len(wins[k])
        s_blocks.append(np.ascontiguousarray(s_all.astype(BF16)))

    return radius, perms, s_blocks


_cached = {}


def _get_program(radius):
    key = ("nc", radius)
    if key not in _cached:
        _cached[key] = _build_program(radius)
    return _cached[key]


def _ensure_axon_hooks_importable():
    """bass_utils imports antenv.axon_hooks when tracing is requested.
    Some containers ship an antenv stub without that module; provide a
    no-op registry so tracing degrades gracefully instead of crashing."""
    import sys
    import types

    try:
        import antenv.axon_hooks  # noqa: F401
    except Exception:
        m = types.ModuleType("antenv.axon_hooks")
        m._hook = None
        m.set_axon_ntff_profile_hook = lambda h: setattr(m, "_hook", h)
        m.get_axon_ntff_profile_hook = lambda: getattr(m, "_hook", None)
        sys.modules["antenv.axon_hooks"] = m


def kernel(correlation: np.ndarray, flow: np.ndarray, _trace: bool = False):
    _ensure_axon_hooks_importable()
    from concourse.bass_utils import run_bass_kernel_spmd

    correlation = np.asarray(correlation, dtype=np.float32)
    flow = np.asarray(flow, dtype=np.float32)

    radius, perms, s_blocks = _plan(flow)

    in_maps = []
    for core in range(N_CORES):
        b, half = divmod(core, 2)
        ch0 = half * CH_PER_CORE
        # band[p, s, ch] = corr[b, s*128+p, ch0+ch]
        band = (
            correlation[b]
            .reshape(HW, HW)[:, ch0 : ch0 + CH_PER_CORE]
            .astype(BF16)
            .reshape(N_PTILES, P, CH_PER_CORE)
            .transpose(1, 0, 2)
        )
        in_maps.append(
            {
                "band": np.ascontiguousarray(band),
                "s_all": s_blocks[b],
            }
        )

    nc = _get_program(radius)
    extra = {"trace_cores": list(range(N_CORES))} if _trace else {}
    res = run_bass_kernel_spmd(
        nc, in_maps, core_ids=list(range(N_CORES)), trace=_trace, **extra
    )

    out = np.empty((B, HW, HW), dtype=np.float32)
    for b in range(B):
        halves = [
            np.asarray(res.results[2 * b + h]["out"]).astype(np.float32)
            for h in range(2)
        ]
        # [P, N_PTILES, 4096] -> sorted-position-major [4096, 4096]
        full = np.concatenate(halves, axis=2).transpose(1, 0, 2).reshape(HW, HW)
        out[b, perms[b], :] = full
    if _trace:
        kernel.last_results = res
    return out.reshape(B, H, W, HW)
